# revision 1
# baseline (speedup 1.0000x reference)
"""Trainium2 Bass kernel for the CustomLossFilter loss.

reference semantics (per row, fp32):
    cond = |inputs[:,4] - inputs[:,2]| < 0.1
    diff = where(cond, inputs[:,0] - inputs[:,4], inputs[:,0] - targets[:,0])
    out  = mean(|diff|)

Strategy: data-parallel over the 20M rows across 8 NeuronCores (2.5M rows
per core).  Inside a core, rows are mapped [128 partitions x 19531 rows]
with each partition owning a contiguous row range, so every DMA is a plain
contiguous 2D transfer.  Columns 0/2/4 are accessed with stride-5 APs in
SBUF.  Each core emits a [128,1] vector of per-partition |diff| sums; the
host adds the 1024 partials and divides by N.
"""

import numpy as np

import concourse.bacc as bacc
import concourse.mybir as mybir
from concourse import tile
from concourse.bass_utils import run_bass_kernel_spmd

N_TOTAL = 20_000_000
F = 5
N_CORES = 8
ROWS = N_TOTAL // N_CORES  # 2_500_000 rows per core
P = 128
W = 2048  # rows per partition per tile
ERR_OK = 0.1

_ALU = mybir.AluOpType
_AX = mybir.AxisListType
_F32 = mybir.dt.float32
_U8 = mybir.dt.uint8
_ABS = mybir.ActivationFunctionType.Abs
_CPY = mybir.ActivationFunctionType.Copy


def _body(tc, inp, tgt, out, rows, w):
    nc = tc.nc
    rpp = rows // P          # rows per partition in the main region
    scrap = rows - P * rpp   # leftover rows (< 128)

    widths = []
    off = 0
    while off < rpp:
        widths.append(min(w, rpp - off))
        off += widths[-1]
    nt = len(widths) + (1 if scrap else 0)

    # [128, rpp*5] / [128, rpp] contiguous-per-partition views of DRAM
    in_main = inp[: P * rpp, :].rearrange("(p r) f -> p (r f)", p=P)
    tg_main = tgt[: P * rpp, :].rearrange("(p r) f -> p (r f)", p=P)

    with (
        tc.tile_pool(name="acc", bufs=1) as accpool,
        tc.tile_pool(name="inp", bufs=3) as inpool,
        tc.tile_pool(name="tgp", bufs=3) as tgpool,
        tc.tile_pool(name="wrk", bufs=3) as wpool,
    ):
        acc = accpool.tile([P, nt], _F32)
        nc.vector.memset(acc[:], 0.0)

        off = 0
        for t, wt in enumerate(widths):
            ti = inpool.tile([P, w * F], _F32, tag="in")
            tt = tgpool.tile([P, w], _F32, tag="tg")
            nc.sync.dma_start(ti[:, : wt * F], in_main[:, off * F : (off + wt) * F])
            nc.scalar.dma_start(tt[:, :wt], tg_main[:, off : off + wt])

            in0 = ti[:, 0 : wt * F : F]
            in2 = ti[:, 2 : wt * F : F]
            in4 = ti[:, 4 : wt * F : F]

            d = wpool.tile([P, w], _F32, tag="d")
            absd = wpool.tile([P, w], _F32, tag="a")
            m = wpool.tile([P, w], _U8, tag="m")
            nc.vector.tensor_tensor(d[:, :wt], in4, in2, _ALU.subtract)
            nc.scalar.activation(absd[:, :wt], d[:, :wt], _ABS)
            nc.vector.tensor_scalar(m[:, :wt], absd[:, :wt], ERR_OK, None, _ALU.is_lt)
            nc.vector.copy_predicated(tt[:, :wt], m[:, :wt], in4)
            diff = wpool.tile([P, w], _F32, tag="d")
            adiff = wpool.tile([P, w], _F32, tag="a")
            nc.vector.tensor_tensor(diff[:, :wt], in0, tt[:, :wt], _ALU.subtract)
            nc.scalar.activation(
                adiff[:, :wt], diff[:, :wt], _ABS, accum_out=acc[:, t : t + 1]
            )
            off += wt

        if scrap:
            si = inpool.tile([scrap, F], _F32, tag="sin")
            st = tgpool.tile([scrap, 1], _F32, tag="stg")
            nc.sync.dma_start(si[:], inp[P * rpp :, :])
            nc.scalar.dma_start(st[:], tgt[P * rpp :, :])
            sd = wpool.tile([scrap, 1], _F32, tag="sd")
            sa = wpool.tile([scrap, 1], _F32, tag="sa")
            sm = wpool.tile([scrap, 1], _U8, tag="sm")
            nc.vector.tensor_tensor(sd[:], si[:, 4:5], si[:, 2:3], _ALU.subtract)
            nc.scalar.activation(sa[:], sd[:], _ABS)
            nc.vector.tensor_scalar(sm[:], sa[:], ERR_OK, None, _ALU.is_lt)
            nc.vector.copy_predicated(st[:], sm[:], si[:, 4:5])
            sdiff = wpool.tile([scrap, 1], _F32, tag="sd")
            sadiff = wpool.tile([scrap, 1], _F32, tag="sa")
            nc.vector.tensor_tensor(sdiff[:], si[:, 0:1], st[:], _ALU.subtract)
            nc.scalar.activation(
                sadiff[:], sdiff[:], _ABS, accum_out=acc[:scrap, nt - 1 : nt]
            )

        res = accpool.tile([P, 1], _F32)
        nc.vector.tensor_reduce(res[:], acc[:], axis=_AX.X, op=_ALU.add)
        nc.sync.dma_start(out[:], res[:])


def build_nc(rows=ROWS, w=W):
    nc = bacc.Bacc(
        "TRN2", target_bir_lowering=False, debug=False, num_devices=N_CORES
    )
    inp = nc.dram_tensor("inputs", [rows, F], _F32, kind="ExternalInput").ap()
    tgt = nc.dram_tensor("targets", [rows, 1], _F32, kind="ExternalInput").ap()
    out = nc.dram_tensor("out", [P, 1], _F32, kind="ExternalOutput").ap()
    with tile.TileContext(nc) as tc:
        _body(tc, inp, tgt, out, rows, w)
    nc.compile()
    return nc


_NC_CACHE = {}


def _get_nc():
    if "nc" not in _NC_CACHE:
        _NC_CACHE["nc"] = build_nc()
    return _NC_CACHE["nc"]


def run_sharded(inputs, targets, **spmd_kwargs):
    """Run the SPMD kernel; returns (per-core [128,1] partials, results obj)."""
    nc = _get_nc()
    inputs = np.asarray(inputs, dtype=np.float32)
    targets = np.asarray(targets, dtype=np.float32)
    in_maps = [
        {
            "inputs": inputs[i * ROWS : (i + 1) * ROWS],
            "targets": targets[i * ROWS : (i + 1) * ROWS],
        }
        for i in range(N_CORES)
    ]
    res = run_bass_kernel_spmd(nc, in_maps, list(range(N_CORES)), **spmd_kwargs)
    partials = np.stack([r["out"] for r in res.results])  # [8, 128, 1]
    return partials, res


def kernel(inputs, targets):
    partials, _ = run_sharded(inputs, targets)
    total = partials.astype(np.float64).sum()
    return np.asarray(total / N_TOTAL, dtype=np.float32)



# revision 2
# speedup vs baseline: 1.3324x; 1.3324x over previous
"""Trainium2 Bass kernel for the CustomLossFilter loss.

reference semantics (per row, fp32):
    cond = |inputs[:,4] - inputs[:,2]| < 0.1
    diff = where(cond, inputs[:,0] - inputs[:,4], inputs[:,0] - targets[:,0])
    out  = mean(|diff|)

Strategy: data-parallel over the 20M rows across 8 NeuronCores (2.5M rows
per core).  The reference only reads input columns 0/2/4, so the host-side
shard step packs exactly those columns (plus targets) into planar per-core
blocks: two [128, 2, RPP] f32 tensors per core holding (c0, c2) and
(c4, tgt) column planes, with each partition owning a contiguous row range
and rows zero-padded to RPP*128.  This cuts DMA traffic from 60MB to 40MB
per core and makes every SBUF operand contiguous (the interleaved stride-5
layout cost the vector engine ~3x per op).  Each core emits a [128,1]
vector of per-partition |diff| sums; the host adds the 1024 partials and
divides by the true N.  Padded rows are zeros in every plane, so they
contribute |0-0| = 0 to the sum.
"""

import numpy as np

import concourse.bacc as bacc
import concourse.mybir as mybir
from concourse import tile
from concourse.bass_utils import run_bass_kernel_spmd

N_TOTAL = 20_000_000
N_CORES = 8
ROWS = N_TOTAL // N_CORES  # 2_500_000 real rows per core
P = 128
RPP = 19_532               # rows per partition (128*19532 = 2_500_096)
PADROWS = P * RPP
W = 2048                   # rows per partition per tile
ERR_OK = 0.1

_ALU = mybir.AluOpType
_AX = mybir.AxisListType
_F32 = mybir.dt.float32
_U8 = mybir.dt.uint8
_ABS = mybir.ActivationFunctionType.Abs


def _body(tc, pa, pb, out):
    nc = tc.nc

    widths = []
    off = 0
    while off < RPP:
        widths.append(min(W, RPP - off))
        off += widths[-1]
    nt = len(widths)

    with (
        tc.tile_pool(name="acc", bufs=1) as accpool,
        tc.tile_pool(name="ina", bufs=4) as apool,
        tc.tile_pool(name="inb", bufs=4) as bpool,
        tc.tile_pool(name="wrk", bufs=2) as wpool,
    ):
        acc = accpool.tile([P, nt], _F32)
        nc.vector.memset(acc[:], 0.0)

        off = 0
        for t, wt in enumerate(widths):
            ta = apool.tile([P, 2, W], _F32, tag="a")
            tb = bpool.tile([P, 2, W], _F32, tag="b")
            nc.sync.dma_start(ta[:, :, :wt], pa[:, :, off : off + wt])
            nc.gpsimd.dma_start(tb[:, :, :wt], pb[:, :, off : off + wt])

            c0 = ta[:, 0, :wt]
            c2 = ta[:, 1, :wt]
            c4 = tb[:, 0, :wt]
            tg = tb[:, 1, :wt]

            d = wpool.tile([P, W], _F32, tag="d")
            a = wpool.tile([P, W], _F32, tag="a")
            m = wpool.tile([P, W], _U8, tag="m")
            d2 = wpool.tile([P, W], _F32, tag="d2")
            a2 = wpool.tile([P, W], _F32, tag="a2")
            nc.vector.tensor_tensor(d[:, :wt], c4, c2, _ALU.subtract)
            nc.scalar.activation(a[:, :wt], d[:, :wt], _ABS)
            nc.vector.tensor_scalar(m[:, :wt], a[:, :wt], ERR_OK, None, _ALU.is_lt)
            nc.vector.copy_predicated(tg, m[:, :wt], c4)
            nc.vector.tensor_tensor(d2[:, :wt], c0, tg, _ALU.subtract)
            nc.scalar.activation(
                a2[:, :wt], d2[:, :wt], _ABS, accum_out=acc[:, t : t + 1]
            )
            off += wt

        res = accpool.tile([P, 1], _F32)
        nc.vector.tensor_reduce(res[:], acc[:], axis=_AX.X, op=_ALU.add)
        nc.sync.dma_start(out[:], res[:])


def build_nc():
    nc = bacc.Bacc(
        "TRN2", target_bir_lowering=False, debug=False, num_devices=N_CORES
    )
    pa = nc.dram_tensor("pa", [P, 2, RPP], _F32, kind="ExternalInput").ap()
    pb = nc.dram_tensor("pb", [P, 2, RPP], _F32, kind="ExternalInput").ap()
    out = nc.dram_tensor("out", [P, 1], _F32, kind="ExternalOutput").ap()
    with tile.TileContext(nc) as tc:
        _body(tc, pa, pb, out)
    nc.compile()
    return nc


_NC_CACHE = {}


def _get_nc():
    if "nc" not in _NC_CACHE:
        _NC_CACHE["nc"] = build_nc()
    return _NC_CACHE["nc"]


def _pack_core(inputs_slice, targets_slice):
    """Pack one core's rows into the two planar [P, 2, RPP] device tensors."""
    pa = np.zeros((P, 2, RPP), dtype=np.float32)
    pb = np.zeros((P, 2, RPP), dtype=np.float32)
    col = np.zeros(PADROWS, dtype=np.float32)
    for dst, plane, src in (
        (pa, 0, inputs_slice[:, 0]),
        (pa, 1, inputs_slice[:, 2]),
        (pb, 0, inputs_slice[:, 4]),
        (pb, 1, targets_slice[:, 0]),
    ):
        col[:ROWS] = src
        dst[:, plane, :] = col.reshape(P, RPP)
    return pa, pb


def run_sharded(inputs, targets, **spmd_kwargs):
    """Run the SPMD kernel; returns (per-core [128,1] partials, results obj)."""
    nc = _get_nc()
    inputs = np.asarray(inputs, dtype=np.float32)
    targets = np.asarray(targets, dtype=np.float32)
    in_maps = []
    for i in range(N_CORES):
        pa, pb = _pack_core(
            inputs[i * ROWS : (i + 1) * ROWS],
            targets[i * ROWS : (i + 1) * ROWS],
        )
        in_maps.append({"pa": pa, "pb": pb})
    res = run_bass_kernel_spmd(nc, in_maps, list(range(N_CORES)), **spmd_kwargs)
    partials = np.stack([r["out"] for r in res.results])  # [8, 128, 1]
    return partials, res


def kernel(inputs, targets):
    partials, _ = run_sharded(inputs, targets)
    total = partials.astype(np.float64).sum()
    return np.asarray(total / N_TOTAL, dtype=np.float32)
